# revision 1
# baseline (speedup 1.0000x reference)
"""Trainium2 Bass kernel for nn_Attn_76424648065726.

Computes softmax(einsum('so,o->s', outputs @ W.T + b, w)) reshaped to
[1, 1, S].

Math: (outputs @ W.T + b) @ w == outputs @ (W.T @ w) + dot(b, w), and the
scalar dot(b, w) cancels inside softmax.  So the kernel computes
softmax(outputs @ v) with v = W.T @ w — turning the [S,H2]x[H2,H2] matmul
into a memory-bound matvec pipeline.

Sharding (8 cores, hidden-dim parallel): core k owns columns
[512k, 512k+512) of both W and outputs.
  phase 1: v_k = W[:, cols_k].T @ w                  (PE, PSUM accumulate)
  phase 2: e_k[s] = outputs[s, cols_k] @ v_k         (DVE scalar_tensor_tensor)
  AllReduce(add) over e_k -> full energies on every core
  softmax on-device (redundant per core), host takes core 0's output.

outputs/W/w are staged to fp16 on the host (halves HBM traffic, 4x PE
rate).  fp16's 11-bit mantissa keeps the energy perturbation ~0.05
absolute (softmax output err ~5e-4); all accumulation is fp32
(PSUM + STT accumulator).  Values are O(1-10): no fp16 range risk.
"""

import numpy as np

N_CORES = 8
S = 8192
H2 = 4096
HS = H2 // N_CORES  # 512 columns of W / outputs per core
N_OCHUNK = H2 // 128  # 32 contraction chunks for v
N_SBLK = S // 128  # 64 row-blocks of outputs per core
WCPG = 4  # o-chunks per W tile (DMA batch)
XBPG = 8  # s-blocks per X tile (DMA batch)

_CACHE = {}


def _build_nc(enable_asserts=False):
    import concourse.bass as bass
    import concourse.tile as tile
    from concourse import bacc, mybir

    nc = bacc.Bacc(
        "TRN2",
        target_bir_lowering=False,
        debug=False,
        enable_asserts=enable_asserts,
        num_devices=N_CORES,
    )
    fp32 = mybir.dt.float32
    f16 = mybir.dt.float16
    x_d = nc.dram_tensor("x", [S, HS], f16, kind="ExternalInput").ap()
    wc_d = nc.dram_tensor("wc", [H2, HS], f16, kind="ExternalInput").ap()
    wt_d = nc.dram_tensor("wt", [128, N_OCHUNK], f16, kind="ExternalInput").ap()
    p_d = nc.dram_tensor("p", [128, N_SBLK], fp32, kind="ExternalOutput").ap()

    with tile.TileContext(nc) as tc:
        _body(tc, x_d, wc_d, wt_d, p_d)
    nc.compile()
    return nc


def _body(tc, x_d, wc_d, wt_d, p_d):
    import concourse.bass as bass
    from concourse import bass_isa, mybir

    nc = tc.nc
    fp32 = mybir.dt.float32
    f16 = mybir.dt.float16
    ts = bass.ts

    from contextlib import ExitStack

    with ExitStack() as ctx:
        wpool = ctx.enter_context(tc.tile_pool(name="wpool", bufs=8))
        xpool = ctx.enter_context(tc.tile_pool(name="xpool", bufs=8))
        spool = ctx.enter_context(tc.tile_pool(name="spool", bufs=6))
        vpsum = ctx.enter_context(tc.tile_pool(name="vpsum", bufs=1, space="PSUM"))
        small = ctx.enter_context(tc.tile_pool(name="small", bufs=1))
        dram = ctx.enter_context(tc.tile_pool(name="dram", bufs=1, space="DRAM"))

        # w, pre-transposed on host to [128, 32]: wt[p, c] = w[c*128 + p]
        wt_sb = small.tile([128, N_OCHUNK], f16)
        nc.scalar.dma_start(wt_sb[:], wt_d[:])

        # ---- phase 1: v = W_k.T @ w  ([1, HS] accumulated in PSUM) ----
        # All streaming DMAs go on the sync HWDGE ring in FIFO order: W
        # first (it gates everything), then X.  One ring keeps HBM busy;
        # spreading across the scalar ring gains nothing (both rings share
        # the same 16 SDMA engines) and ring backpressure would stall ACT.
        # ~1MiB slices amortize the ~1us per-descriptor ring overhead.
        wtiles = []
        for g in range(N_OCHUNK // WCPG):
            wtile = wpool.tile([128, WCPG, HS], f16)
            nc.sync.dma_start(
                wtile[:],
                wc_d[ts(g, 128 * WCPG), :].rearrange("(c p) j -> p c j", p=128),
            )
            wtiles.append(wtile)

        xtiles = []
        for g in range(N_SBLK // XBPG):
            xt = xpool.tile([128, XBPG, HS], f16)
            nc.sync.dma_start(
                xt[:],
                x_d[ts(g, 128 * XBPG), :].rearrange("(u p) j -> p u j", p=128),
            )
            xtiles.append(xt)

        # PE warmup: the HAM throttles a cold PE to 1.2 GHz; ~10us of dummy
        # matmuls on memset data while W streams in gets the real matmuls
        # the 2.4 GHz rate (480 -> ~240ns each).
        wu_pool = ctx.enter_context(tc.tile_pool(name="wu_pool", bufs=1))
        wu_psum = ctx.enter_context(tc.tile_pool(name="wu_psum", bufs=1, space="PSUM"))
        wu_lhs = wu_pool.tile([128, 1], f16)
        wu_rhs = wu_pool.tile([128, HS], f16)
        nc.vector.memset(wu_lhs[:], 0.0)
        nc.vector.memset(wu_rhs[:], 0.0)
        wu_ps = wu_psum.tile([1, HS], fp32)
        for i in range(10):
            nc.tensor.matmul(
                wu_ps[:], lhsT=wu_lhs[:], rhs=wu_rhs[:], start=True, stop=True
            )
        # short (N=128) dummies bridge the gap until W arrives — a ~2us PE
        # idle re-throttles the HAM, and they drain fast once real work is
        # ready.
        for i in range(20):
            nc.tensor.matmul(
                wu_ps[:, :128], lhsT=wu_lhs[:], rhs=wu_rhs[:, :128],
                start=True, stop=True,
            )

        v_ps = vpsum.tile([1, HS], fp32)
        for c in range(N_OCHUNK):
            nc.tensor.matmul(
                v_ps[:],
                lhsT=wt_sb[:, c : c + 1],
                rhs=wtiles[c // WCPG][:, c % WCPG, :],
                start=(c == 0),
                stop=(c == N_OCHUNK - 1),
            )

        v_row = small.tile([1, HS], f16)
        nc.vector.tensor_copy(v_row[:], v_ps[:])
        vb = small.tile([128, HS], f16)
        nc.gpsimd.partition_broadcast(vb[:], v_row[:])

        # ---- phase 2: partial energies e_sb[p, b] = X[128b+p, :] @ v_k ----
        # Single AllReduce at the end: in barrier-bound runs (the common
        # case here) a split cascade pays pickup+duration twice after the
        # barrier (~35us) while a single 32KiB AR pays once (~18us); the
        # split only wins when the entry barrier is short.
        e_sb = small.tile([128, N_SBLK], fp32)
        e_dr = dram.tile([128, N_SBLK], fp32)
        e_red = dram.tile([128, N_SBLK], fp32)

        # Lane balance: fused STT on DVE costs ~772ns/block; offloaded
        # blocks cost DVE ~425ns (2x-mode tensor_mul) + ACT ~985ns
        # (accumulate + read-accumulator).  Offloading 37 of 64 equalizes
        # the two lanes at ~36us (vs 49us DVE-only).
        OFFLOAD = 37

        def is_offload(b):
            return (b * OFFLOAD) // N_SBLK != ((b - 1) * OFFLOAD) // N_SBLK

        for g in range(N_SBLK // XBPG):
            for u in range(XBPG):
                b = g * XBPG + u
                scr = spool.tile([128, HS], f16)
                if is_offload(b):
                    nc.vector.tensor_mul(scr[:], xtiles[g][:, u, :], vb[:])
                    scr2 = spool.tile([128, HS], f16)
                    nc.scalar.activation(
                        scr2[:],
                        scr[:],
                        mybir.ActivationFunctionType.Copy,
                        bias=0.0,
                        scale=1.0,
                        accum_out=e_sb[:, b : b + 1],
                    )
                else:
                    nc.vector.scalar_tensor_tensor(
                        out=scr[:],
                        in0=xtiles[g][:, u, :],
                        scalar=1.0,
                        in1=vb[:],
                        op0=mybir.AluOpType.mult,
                        op1=mybir.AluOpType.mult,
                        accum_out=e_sb[:, b : b + 1],
                    )

        # feed + trigger on the empty scalar ring / idle gpsimd
        nc.scalar.dma_start(e_dr[:], e_sb[:])
        nc.gpsimd.collective_compute(
            "AllReduce",
            mybir.AluOpType.add,
            replica_groups=[list(range(N_CORES))],
            ins=[e_dr.opt()],
            outs=[e_red.opt()],
        )
        ef = small.tile([128, N_SBLK], fp32)
        nc.scalar.dma_start(ef[:], e_red[:])

        # ---- softmax over all S values (redundant on every core) ----
        m1 = small.tile([128, 1], fp32)
        nc.vector.tensor_reduce(
            m1[:], ef[:], axis=mybir.AxisListType.X, op=mybir.AluOpType.max
        )
        mb = small.tile([128, 1], fp32)
        nc.gpsimd.partition_all_reduce(
            mb[:], m1[:], channels=128, reduce_op=bass_isa.ReduceOp.max
        )
        nmb = small.tile([128, 1], fp32)
        nc.scalar.mul(nmb[:], mb[:], -1.0)
        pexp = small.tile([128, N_SBLK], fp32)
        s1 = small.tile([128, 1], fp32)
        nc.scalar.activation(
            pexp[:],
            ef[:],
            mybir.ActivationFunctionType.Exp,
            bias=nmb[:],
            scale=1.0,
            accum_out=s1[:],
        )
        zb = small.tile([128, 1], fp32)
        nc.gpsimd.partition_all_reduce(
            zb[:], s1[:], channels=128, reduce_op=bass_isa.ReduceOp.add
        )
        rz = small.tile([128, 1], fp32)
        nc.vector.reciprocal(rz[:], zb[:])
        po = small.tile([128, N_SBLK], fp32)
        nc.scalar.mul(po[:], pexp[:], rz[:])
        nc.scalar.dma_start(p_d[:], po[:])


def _shard_inputs(outputs, W, w):
    # per-shard slice-copies beat a bulk transpose-copy here (measured
    # 252ms vs 444ms: contiguous writes win over strided ones)
    f16 = np.float16
    outputs = np.asarray(outputs, dtype=np.float32)
    W = np.asarray(W, dtype=np.float32)
    w = np.asarray(w, dtype=np.float32)
    wt = np.ascontiguousarray(w.reshape(N_OCHUNK, 128).T).astype(f16)
    in_maps = []
    for k in range(N_CORES):
        cols = slice(HS * k, HS * (k + 1))
        in_maps.append(
            {
                "x": np.ascontiguousarray(outputs[:, cols]).astype(f16),
                "wc": np.ascontiguousarray(W[:, cols]).astype(f16),
                "wt": wt,
            }
        )
    return in_maps


def _run(outputs, W, w, trace=False, trace_cores=None):
    from concourse.bass_utils import run_bass_kernel_spmd

    if "nc" not in _CACHE:
        _CACHE["nc"] = _build_nc()
    nc = _CACHE["nc"]
    in_maps = _shard_inputs(outputs, W, w)
    res = run_bass_kernel_spmd(
        nc, in_maps, list(range(N_CORES)), trace=trace, trace_cores=trace_cores
    )
    p = res.results[0]["p"]  # [128, 64]; full[s = c*128 + p] = p[p, c]
    full = np.ascontiguousarray(p.T).reshape(1, 1, S).astype(np.float32)
    return full, res


def kernel(outputs, W, b, w):
    out, _ = _run(outputs, W, w, trace=False)
    return out


def kernel_traced(outputs, W, b, w, trace_cores=None):
    out, res = _run(outputs, W, w, trace=True, trace_cores=trace_cores)
    return out, res



# revision 8
# speedup vs baseline: 1.1764x; 1.1764x over previous
"""Trainium2 Bass kernel for nn_Attn_76424648065726.

Computes softmax(einsum('so,o->s', outputs @ W.T + b, w)) reshaped to
[1, 1, S].

Math: (outputs @ W.T + b) @ w == outputs @ (W.T @ w) + dot(b, w), and the
scalar dot(b, w) cancels inside softmax.  So the kernel computes
softmax(outputs @ v) with v = W.T @ w — turning the [S,H2]x[H2,H2] matmul
into a memory-bound matvec pipeline.

Sharding (8 cores, hidden-dim parallel): core k owns columns
[512k, 512k+512) of both W and outputs (no cross-core data needed until
the energies are summed).
  phase 1: v_k = W[:, cols_k].T @ w            (PE, PSUM accumulate)
  PE-transpose v_k [1,512] -> vt [128,4]
  phase 2: e_k[s] = outputs[s, cols_k] @ v_k   (PE matvec on X^T tiles)
  ReduceScatter(add): core k gets summed energies for s in
    [1024k, 1024k+1024)
  local max/exp/sum, AllGather of the (max, expsum) pairs (8 B/rank),
  rescale own 1024 values, output own chunk; host concatenates.

outputs/W/w are staged to fp16 on the host (halves HBM traffic, faster
PE).  All accumulation is fp32 (PSUM / ACT accumulator).  X is staged
host-side in a transposed, DMA-friendly tiled layout so phase 2 runs on
the otherwise-idle PE instead of DVE+ACT.
"""

import numpy as np

N_CORES = 8
S = 8192
H2 = 4096
HS = H2 // N_CORES  # 512 columns of W / outputs per core
N_OCHUNK = H2 // 128  # 32 contraction chunks for v
WCPG = 4  # o-chunks per W tile (DMA batch)
NHC = HS // 128  # 4 h-chunks per core
NG = 8  # X s-groups per core (1024 s each)
SG = S // NG  # 1024
SC = S // N_CORES  # 1024 output chunk per core

_CACHE = {}


def _build_nc(enable_asserts=False, debug_taps=False):
    import concourse.bass as bass
    import concourse.tile as tile
    from concourse import bacc, mybir

    nc = bacc.Bacc(
        "TRN2",
        target_bir_lowering=False,
        debug=False,
        enable_asserts=enable_asserts,
        num_devices=N_CORES,
    )
    fp32 = mybir.dt.float32
    f16 = mybir.dt.float16
    # x: X[:, cols_k] transposed + grouped:
    #   x[(g*4 + hc)*128 + p, s] = X[1024g + s, 512k + 128hc + p]
    x_d = nc.dram_tensor("x", [NG * HS, SG], f16, kind="ExternalInput").ap()
    wc_d = nc.dram_tensor("wc", [H2, HS], f16, kind="ExternalInput").ap()
    wt_d = nc.dram_tensor("wt", [128, N_OCHUNK], f16, kind="ExternalInput").ap()
    p_d = nc.dram_tensor("p", [1, SC], fp32, kind="ExternalOutput").ap()
    dbg = None
    if debug_taps:
        dbg = {
            "dbg_vt": nc.dram_tensor("dbg_vt", [128, NHC], f16,
                                     kind="ExternalOutput").ap(),
            "dbg_e": nc.dram_tensor("dbg_e", [1, S], fp32,
                                    kind="ExternalOutput").ap(),
            "dbg_ef": nc.dram_tensor("dbg_ef", [1, SC], fp32,
                                     kind="ExternalOutput").ap(),
            "dbg_st": nc.dram_tensor("dbg_st", [N_CORES, 2], fp32,
                                     kind="ExternalOutput").ap(),
        }

    with tile.TileContext(nc) as tc:
        _body(tc, x_d, wc_d, wt_d, p_d, dbg=dbg)
    nc.compile()
    return nc


def _body(tc, x_d, wc_d, wt_d, p_d, dbg=None):
    import concourse.bass as bass
    from concourse import bass_isa, mybir

    nc = tc.nc
    fp32 = mybir.dt.float32
    f16 = mybir.dt.float16
    ts = bass.ts

    from contextlib import ExitStack

    with ExitStack() as ctx:
        wpool = ctx.enter_context(tc.tile_pool(name="wpool", bufs=8))
        xpool = ctx.enter_context(tc.tile_pool(name="xpool", bufs=8))
        small = ctx.enter_context(tc.tile_pool(name="small", bufs=1))
        dram = ctx.enter_context(tc.tile_pool(name="dram", bufs=1, space="DRAM"))

        # w, pre-transposed on host to [128, 32]: wt[p, c] = w[c*128 + p]
        wt_sb = small.tile([128, N_OCHUNK], f16)
        nc.scalar.dma_start(wt_sb[:], wt_d[:])

        # All streaming DMAs go on the sync HWDGE ring in FIFO order: W
        # first (it gates phase 1), then X.  One ring keeps HBM busy;
        # both rings share the same 16 SDMA engines so spreading gains
        # nothing.  ~0.5-1MiB slices amortize ring overhead.
        wtiles = []
        for g in range(N_OCHUNK // WCPG):
            wtile = wpool.tile([128, WCPG, HS], f16)
            nc.sync.dma_start(
                wtile[:],
                wc_d[ts(g, 128 * WCPG), :].rearrange("(c p) j -> p c j", p=128),
            )
            wtiles.append(wtile)

        xtiles = []
        for g in range(NG):
            xt = xpool.tile([128, NHC, SG], f16)
            nc.sync.dma_start(
                xt[:],
                x_d[ts(g, NHC * 128), :].rearrange("(h p) s -> p h s", p=128),
            )
            xtiles.append(xt)

        with ExitStack() as p1ctx:
            # PE warmup: the HAM throttles a cold PE to 1.2 GHz; ~10us of
            # dummy matmuls on memset data while W streams in gets the real
            # matmuls the 2.4 GHz rate.
            wu_pool = p1ctx.enter_context(tc.tile_pool(name="wu_pool", bufs=1))
            wu_psum = p1ctx.enter_context(
                tc.tile_pool(name="wu_psum", bufs=1, space="PSUM")
            )
            vpsum = p1ctx.enter_context(
                tc.tile_pool(name="vpsum", bufs=1, space="PSUM")
            )
            vtpsum = p1ctx.enter_context(
                tc.tile_pool(name="vtpsum", bufs=1, space="PSUM")
            )
            wu_lhs = wu_pool.tile([128, 1], f16)
            wu_rhs = wu_pool.tile([128, HS], f16)
            nc.vector.memset(wu_lhs[:], 0.0)
            nc.vector.memset(wu_rhs[:], 0.0)
            wu_ps = wu_psum.tile([1, HS], fp32)
            for i in range(10):
                nc.tensor.matmul(
                    wu_ps[:], lhsT=wu_lhs[:], rhs=wu_rhs[:], start=True, stop=True
                )
            # short (N=128) dummies bridge the gap until W arrives — a ~2us
            # PE idle re-throttles the HAM.
            for i in range(20):
                nc.tensor.matmul(
                    wu_ps[:, :128], lhsT=wu_lhs[:], rhs=wu_rhs[:, :128],
                    start=True, stop=True,
                )

            # ---- phase 1: v = W_k.T @ w  ([1, HS] accumulated in PSUM) ----
            v_ps = vpsum.tile([1, HS], fp32)
            for c in range(N_OCHUNK):
                nc.tensor.matmul(
                    v_ps[:],
                    lhsT=wt_sb[:, c : c + 1],
                    rhs=wtiles[c // WCPG][:, c % WCPG, :],
                    start=(c == 0),
                    stop=(c == N_OCHUNK - 1),
                )

            v_row = small.tile([1, HS], fp32)
            nc.vector.tensor_copy(v_row[:], v_ps[:])

            # PE-transpose v [1, 512] -> vt [128, 4] (vt[p, hc] = v[128hc+p])
            # fp32 keeps the PSUM column slices 4-byte aligned.
            id1 = wu_pool.tile([1, 1], fp32)
            nc.vector.memset(id1[:], 1.0)
            vt_ps = vtpsum.tile([128, NHC], fp32)
            for hc in range(NHC):
                nc.tensor.transpose(
                    vt_ps[:, hc : hc + 1], v_row[:, ts(hc, 128)], id1[:]
                )
            vt = small.tile([128, NHC], f16)
            nc.vector.tensor_copy(vt[:], vt_ps[:])
            if dbg is not None:
                nc.scalar.dma_start(dbg["dbg_vt"][:], vt[:])

        # ---- phase 2: partial energies e[s] = X[s, cols_k] @ v_k on PE ----
        # e lives as [1, 8192] fp32 on partition 0.
        e_sb = small.tile([1, S], fp32)
        with ExitStack() as p2ctx:
            epool = p2ctx.enter_context(
                tc.tile_pool(name="epsum", bufs=8, space="PSUM")
            )
            for g in range(NG):
                for half in range(2):
                    e_ps = epool.tile([1, 512], fp32)
                    for hc in range(NHC):
                        nc.tensor.matmul(
                            e_ps[:],
                            lhsT=vt[:, hc : hc + 1],
                            rhs=xtiles[g][:, hc, ts(half, 512)],
                            start=(hc == 0),
                            stop=(hc == NHC - 1),
                        )
                    nc.vector.tensor_copy(
                        e_sb[:, g * SG + half * 512 : g * SG + half * 512 + 512],
                        e_ps[:],
                    )

        # ---- ReduceScatter: core k gets summed energies for its s-chunk --
        e_dr = dram.tile([N_CORES, SC], fp32)
        e_red = dram.tile([1, SC], fp32)
        for r in range(N_CORES):
            nc.scalar.dma_start(e_dr[r : r + 1, :], e_sb[:, ts(r, SC)])
        nc.gpsimd.collective_compute(
            "ReduceScatter",
            mybir.AluOpType.add,
            replica_groups=[list(range(N_CORES))],
            ins=[e_dr.opt()],
            outs=[e_red.opt()],
        )
        ef = small.tile([1, SC], fp32)
        nc.scalar.dma_start(ef[:], e_red[:])
        if dbg is not None:
            nc.scalar.dma_start(dbg["dbg_e"][:], e_sb[:])
            nc.scalar.dma_start(dbg["dbg_ef"][:], ef[:])

        # ---- local softmax stats on own 1024 energies ----
        m = small.tile([1, 1], fp32)
        nc.vector.tensor_reduce(
            m[:], ef[:], axis=mybir.AxisListType.X, op=mybir.AluOpType.max
        )
        nm = small.tile([1, 1], fp32)
        nc.scalar.mul(nm[:], m[:], -1.0)
        pexp = small.tile([1, SC], fp32)
        s1 = small.tile([1, 1], fp32)
        nc.scalar.activation(
            pexp[:],
            ef[:],
            mybir.ActivationFunctionType.Exp,
            bias=nm[:],
            scale=1.0,
            accum_out=s1[:],
        )

        # ---- AllGather the (max, expsum) pairs: 8 B per rank ----
        st = small.tile([1, 2], fp32)
        nc.vector.tensor_copy(st[:, 0:1], m[:])
        nc.vector.tensor_copy(st[:, 1:2], s1[:])
        st_dr = dram.tile([1, 2], fp32)
        st_all = dram.tile([N_CORES, 2], fp32)
        nc.scalar.dma_start(st_dr[:], st[:])
        nc.gpsimd.collective_compute(
            "AllGather",
            mybir.AluOpType.bypass,
            replica_groups=[list(range(N_CORES))],
            ins=[st_dr.opt()],
            outs=[st_all.opt()],
        )
        if dbg is not None:
            sta = small.tile([N_CORES, 2], fp32)
            nc.scalar.dma_start(sta[:], st_all[:])
            nc.scalar.dma_start(dbg["dbg_st"][:], sta[:])
        # gather columns: sam [1, 8] = maxes, sas [1, 8] = expsums
        sam = small.tile([1, N_CORES], fp32)
        sas = small.tile([1, N_CORES], fp32)
        nc.scalar.dma_start(sam[:], st_all[:, 0:1].rearrange("r c -> c r"))
        nc.scalar.dma_start(sas[:], st_all[:, 1:2].rearrange("r c -> c r"))

        # global M, Z; final scale = exp(m_k - M) / Z
        M = small.tile([1, 1], fp32)
        nc.vector.tensor_reduce(
            M[:], sam[:], axis=mybir.AxisListType.X, op=mybir.AluOpType.max
        )
        nM = small.tile([1, 1], fp32)
        nc.scalar.mul(nM[:], M[:], -1.0)
        sh = small.tile([1, N_CORES], fp32)
        zt = small.tile([1, 1], fp32)
        nc.scalar.activation(
            sh[:],
            sam[:],
            mybir.ActivationFunctionType.Exp,
            bias=nM[:],
            scale=1.0,
        )
        terms = small.tile([1, N_CORES], fp32)
        nc.vector.tensor_mul(terms[:], sh[:], sas[:])
        Z = small.tile([1, 1], fp32)
        nc.vector.tensor_reduce(
            Z[:], terms[:], axis=mybir.AxisListType.X, op=mybir.AluOpType.add
        )
        rz = small.tile([1, 1], fp32)
        nc.vector.reciprocal(rz[:], Z[:])
        esh = small.tile([1, 1], fp32)
        nc.scalar.activation(
            esh[:], m[:], mybir.ActivationFunctionType.Exp, bias=nM[:], scale=1.0
        )
        scale = small.tile([1, 1], fp32)
        nc.vector.tensor_mul(scale[:], esh[:], rz[:])

        po = small.tile([1, SC], fp32)
        nc.scalar.mul(po[:], pexp[:], scale[:])
        nc.scalar.dma_start(p_d[:], po[:])


def _shard_inputs(outputs, W, w):
    f16 = np.float16
    outputs = np.asarray(outputs, dtype=np.float32)
    W = np.asarray(W, dtype=np.float32)
    w = np.asarray(w, dtype=np.float32)
    wt = np.ascontiguousarray(w.reshape(N_OCHUNK, 128).T).astype(f16)
    in_maps = []
    for k in range(N_CORES):
        cols = slice(HS * k, HS * (k + 1))
        xk = outputs[:, cols]  # [8192, 512]
        # [(g, hc*128+p), s] = X[1024g + s, 512k + 128hc + p]
        xt = (
            np.ascontiguousarray(xk.reshape(NG, SG, HS).transpose(0, 2, 1))
            .reshape(NG * HS, SG)
            .astype(f16)
        )
        in_maps.append(
            {
                "x": xt,
                "wc": np.ascontiguousarray(W[:, cols]).astype(f16),
                "wt": wt,
            }
        )
    return in_maps


def _run(outputs, W, w, trace=False, trace_cores=None):
    from concourse.bass_utils import run_bass_kernel_spmd

    if "nc" not in _CACHE:
        _CACHE["nc"] = _build_nc()
    nc = _CACHE["nc"]
    in_maps = _shard_inputs(outputs, W, w)
    res = run_bass_kernel_spmd(
        nc, in_maps, list(range(N_CORES)), trace=trace, trace_cores=trace_cores
    )
    full = np.concatenate(
        [res.results[k]["p"][0] for k in range(N_CORES)]
    ).reshape(1, 1, S).astype(np.float32)
    return full, res


def kernel(outputs, W, b, w):
    out, _ = _run(outputs, W, w, trace=False)
    return out


def kernel_traced(outputs, W, b, w, trace_cores=None):
    out, res = _run(outputs, W, w, trace=True, trace_cores=trace_cores)
    return out, res


# revision 13
# speedup vs baseline: 1.1973x; 1.0177x over previous
"""Trainium2 Bass kernel for nn_Attn_76424648065726.

Computes softmax(einsum('so,o->s', outputs @ W.T + b, w)) reshaped to
[1, 1, S].

Math: (outputs @ W.T + b) @ w == outputs @ (W.T @ w) + dot(b, w), and the
scalar dot(b, w) cancels inside softmax.  So the kernel computes
softmax(outputs @ v) with v = W.T @ w — turning the [S,H2]x[H2,H2] matmul
into a memory-bound matvec pipeline.

Sharding (8 cores, hidden-dim parallel): core k owns columns
[512k, 512k+512) of both W and outputs (no cross-core data needed until
the energies are summed).
  phase 1: v_k = W[:, cols_k].T @ w            (PE, PSUM accumulate)
  PE-transpose v_k [1,512] -> vt [128,4]
  phase 2: e_k[s] = outputs[s, cols_k] @ v_k   (PE matvec on X^T tiles)
  ReduceScatter(add): core k gets summed energies for s in
    [1024k, 1024k+1024)
  local max/exp/sum, AllGather of the (max, expsum) pairs (8 B/rank),
  rescale own 1024 values, output own chunk; host concatenates.

outputs/W/w are staged to fp16 on the host (halves HBM traffic, faster
PE).  All accumulation is fp32 (PSUM / ACT accumulator).  X is staged
host-side in a transposed, DMA-friendly tiled layout so phase 2 runs on
the otherwise-idle PE instead of DVE+ACT.
"""

import numpy as np

N_CORES = 8
S = 8192
H2 = 4096
HS = H2 // N_CORES  # 512 columns of W / outputs per core
N_OCHUNK = H2 // 128  # 32 contraction chunks for v
WCPG = 4  # o-chunks per W tile (DMA batch)
NHC = HS // 128  # 4 h-chunks per core
NG = 8  # X s-groups per core (1024 s each)
SG = S // NG  # 1024
SC = S // N_CORES  # 1024 output chunk per core

_CACHE = {}


def _build_nc(enable_asserts=False, debug_taps=False):
    import concourse.bass as bass
    import concourse.tile as tile
    from concourse import bacc, mybir

    nc = bacc.Bacc(
        "TRN2",
        target_bir_lowering=False,
        debug=False,
        enable_asserts=enable_asserts,
        num_devices=N_CORES,
    )
    fp32 = mybir.dt.float32
    f16 = mybir.dt.float16
    # x: X[:, cols_k] transposed + grouped:
    #   x[(g*4 + hc)*128 + p, s] = X[1024g + s, 512k + 128hc + p]
    x_d = nc.dram_tensor("x", [NG * HS, SG], f16, kind="ExternalInput").ap()
    wc_d = nc.dram_tensor("wc", [H2, HS], f16, kind="ExternalInput").ap()
    wt_d = nc.dram_tensor("wt", [128, N_OCHUNK], f16, kind="ExternalInput").ap()
    p_d = nc.dram_tensor("p", [128, S // 128], fp32, kind="ExternalOutput").ap()
    dbg = None
    if debug_taps:
        dbg = {
            "dbg_vt": nc.dram_tensor("dbg_vt", [128, NHC], f16,
                                     kind="ExternalOutput").ap(),
            "dbg_e": nc.dram_tensor("dbg_e", [1, S], fp32,
                                    kind="ExternalOutput").ap(),
            "dbg_ef": nc.dram_tensor("dbg_ef", [128, 8], fp32,
                                     kind="ExternalOutput").ap(),
        }

    with tile.TileContext(nc) as tc:
        _body(tc, x_d, wc_d, wt_d, p_d, dbg=dbg)
    nc.compile()
    return nc


def _body(tc, x_d, wc_d, wt_d, p_d, dbg=None):
    import concourse.bass as bass
    from concourse import bass_isa, mybir

    nc = tc.nc
    fp32 = mybir.dt.float32
    f16 = mybir.dt.float16
    ts = bass.ts

    from contextlib import ExitStack

    with ExitStack() as ctx:
        wpool = ctx.enter_context(tc.tile_pool(name="wpool", bufs=8))
        xpool = ctx.enter_context(tc.tile_pool(name="xpool", bufs=8))
        small = ctx.enter_context(tc.tile_pool(name="small", bufs=1))
        dram = ctx.enter_context(tc.tile_pool(name="dram", bufs=1, space="DRAM"))

        # w, pre-transposed on host to [128, 32]: wt[p, c] = w[c*128 + p]
        wt_sb = small.tile([128, N_OCHUNK], f16)
        nc.scalar.dma_start(wt_sb[:], wt_d[:])

        # constants for the PE-based partition reduce/broadcast in the
        # softmax tail; built on idle engines during the DMA shadow.
        from concourse import masks

        id128 = small.tile([128, 128], fp32)
        masks.make_identity(nc, id128[:])
        ones_r = small.tile([1, 128], fp32)
        nc.vector.memset(ones_r[:], 1.0)
        ones_c = small.tile([128, 1], fp32)
        nc.vector.memset(ones_c[:], 1.0)

        # All streaming DMAs go on the sync HWDGE ring in FIFO order: W
        # first (it gates phase 1), then X.  One ring keeps HBM busy;
        # both rings share the same 16 SDMA engines so spreading gains
        # nothing.  ~0.5-1MiB slices amortize ring overhead.
        wtiles = []
        for g in range(N_OCHUNK // WCPG):
            wtile = wpool.tile([128, WCPG, HS], f16)
            nc.sync.dma_start(
                wtile[:],
                wc_d[ts(g, 128 * WCPG), :].rearrange("(c p) j -> p c j", p=128),
            )
            wtiles.append(wtile)

        xtiles = []
        for g in range(NG):
            xt = xpool.tile([128, NHC, SG], f16)
            nc.sync.dma_start(
                xt[:],
                x_d[ts(g, NHC * 128), :].rearrange("(h p) s -> p h s", p=128),
            )
            xtiles.append(xt)

        with ExitStack() as p1ctx:
            # PE warmup: the HAM throttles a cold PE to 1.2 GHz; ~10us of
            # dummy matmuls on memset data while W streams in gets the real
            # matmuls the 2.4 GHz rate.
            wu_pool = p1ctx.enter_context(tc.tile_pool(name="wu_pool", bufs=1))
            wu_psum = p1ctx.enter_context(
                tc.tile_pool(name="wu_psum", bufs=1, space="PSUM")
            )
            vpsum = p1ctx.enter_context(
                tc.tile_pool(name="vpsum", bufs=1, space="PSUM")
            )
            vtpsum = p1ctx.enter_context(
                tc.tile_pool(name="vtpsum", bufs=1, space="PSUM")
            )
            wu_lhs = wu_pool.tile([128, 1], f16)
            wu_rhs = wu_pool.tile([128, HS], f16)
            nc.vector.memset(wu_lhs[:], 0.0)
            nc.vector.memset(wu_rhs[:], 0.0)
            wu_ps = wu_psum.tile([1, HS], fp32)
            for i in range(10):
                nc.tensor.matmul(
                    wu_ps[:], lhsT=wu_lhs[:], rhs=wu_rhs[:], start=True, stop=True
                )
            # short (N=128) dummies bridge the gap until W arrives — a ~2us
            # PE idle re-throttles the HAM.
            for i in range(20):
                nc.tensor.matmul(
                    wu_ps[:, :128], lhsT=wu_lhs[:], rhs=wu_rhs[:, :128],
                    start=True, stop=True,
                )

            # ---- phase 1: v = W_k.T @ w  ([1, HS] accumulated in PSUM) ----
            v_ps = vpsum.tile([1, HS], fp32)
            for c in range(N_OCHUNK):
                nc.tensor.matmul(
                    v_ps[:],
                    lhsT=wt_sb[:, c : c + 1],
                    rhs=wtiles[c // WCPG][:, c % WCPG, :],
                    start=(c == 0),
                    stop=(c == N_OCHUNK - 1),
                )

            v_row = small.tile([1, HS], fp32)
            nc.vector.tensor_copy(v_row[:], v_ps[:])

            # PE-transpose v [1, 512] -> vt [128, 4] (vt[p, hc] = v[128hc+p])
            # fp32 keeps the PSUM column slices 4-byte aligned.
            id1 = wu_pool.tile([1, 1], fp32)
            nc.vector.memset(id1[:], 1.0)
            vt_ps = vtpsum.tile([128, NHC], fp32)
            for hc in range(NHC):
                nc.tensor.transpose(
                    vt_ps[:, hc : hc + 1], v_row[:, ts(hc, 128)], id1[:]
                )
            vt = small.tile([128, NHC], f16)
            nc.vector.tensor_copy(vt[:], vt_ps[:])
            if dbg is not None:
                nc.scalar.dma_start(dbg["dbg_vt"][:], vt[:])

        # ---- phase 2: partial energies e[s] = X[s, cols_k] @ v_k on PE ----
        # e lives as [1, 8192] fp32 on partition 0.
        e_sb = small.tile([1, S], fp32)
        with ExitStack() as p2ctx:
            epool = p2ctx.enter_context(
                tc.tile_pool(name="epsum", bufs=8, space="PSUM")
            )
            for g in range(NG):
                for half in range(2):
                    e_ps = epool.tile([1, 512], fp32)
                    for hc in range(NHC):
                        nc.tensor.matmul(
                            e_ps[:],
                            lhsT=vt[:, hc : hc + 1],
                            rhs=xtiles[g][:, hc, ts(half, 512)],
                            start=(hc == 0),
                            stop=(hc == NHC - 1),
                        )
                    nc.vector.tensor_copy(
                        e_sb[:, g * SG + half * 512 : g * SG + half * 512 + 512],
                        e_ps[:],
                    )

        # ---- single AllReduce of the 32 KiB energy vector ----
        e_dr = dram.tile([1, S], fp32)
        e_red = dram.tile([1, S], fp32)
        nc.scalar.dma_start(e_dr[:], e_sb[:])
        nc.gpsimd.collective_compute(
            "AllReduce",
            mybir.AluOpType.add,
            replica_groups=[list(range(N_CORES))],
            ins=[e_dr.opt()],
            outs=[e_red.opt()],
        )
        # land the summed energies partition-spread: ef128[p, c] = e[64p + c]
        ef128 = small.tile([128, S // 128], fp32)
        nc.scalar.dma_start(
            ef128[:], e_red[:].rearrange("o (p c) -> (o p) c", p=128)
        )
        if dbg is not None:
            nc.scalar.dma_start(dbg["dbg_e"][:], e_sb[:])
            nc.scalar.dma_start(dbg["dbg_ef"][:], ef128[:, :8])

        # ---- softmax over all S values (redundant on every core) ----
        # partition reduces / broadcasts run on the idle PE via matmuls.
        with ExitStack() as tctx:
            tpsum = tctx.enter_context(
                tc.tile_pool(name="tpsum", bufs=1, space="PSUM")
            )
            m1 = small.tile([128, 1], fp32)
            nc.vector.tensor_reduce(
                m1[:], ef128[:], axis=mybir.AxisListType.X, op=mybir.AluOpType.max
            )
            mT_ps = tpsum.tile([1, 128], fp32)
            nc.tensor.transpose(mT_ps[:], m1[:], id128[:])
            mT = small.tile([1, 128], fp32)
            nc.vector.tensor_copy(mT[:], mT_ps[:])
            M = small.tile([1, 1], fp32)
            nc.vector.tensor_reduce(
                M[:], mT[:], axis=mybir.AxisListType.X, op=mybir.AluOpType.max
            )
            nM = small.tile([1, 1], fp32)
            nc.scalar.mul(nM[:], M[:], -1.0)
            nmb_ps = tpsum.tile([128, 1], fp32)
            nc.tensor.matmul(
                nmb_ps[:], lhsT=ones_r[:], rhs=nM[:], start=True, stop=True
            )
            nmb = small.tile([128, 1], fp32)
            nc.vector.tensor_copy(nmb[:], nmb_ps[:])

            pexp = small.tile([128, S // 128], fp32)
            s1 = small.tile([128, 1], fp32)
            nc.scalar.activation(
                pexp[:],
                ef128[:],
                mybir.ActivationFunctionType.Exp,
                bias=nmb[:],
                scale=1.0,
                accum_out=s1[:],
            )
            z_ps = tpsum.tile([1, 1], fp32)
            nc.tensor.matmul(
                z_ps[:], lhsT=s1[:], rhs=ones_c[:], start=True, stop=True
            )
            z = small.tile([1, 1], fp32)
            nc.vector.tensor_copy(z[:], z_ps[:])
            rz = small.tile([1, 1], fp32)
            nc.vector.reciprocal(rz[:], z[:])
            rzb_ps = tpsum.tile([128, 1], fp32)
            nc.tensor.matmul(
                rzb_ps[:], lhsT=ones_r[:], rhs=rz[:], start=True, stop=True
            )
            rzb = small.tile([128, 1], fp32)
            nc.vector.tensor_copy(rzb[:], rzb_ps[:])

            po = small.tile([128, S // 128], fp32)
            nc.scalar.mul(po[:], pexp[:], rzb[:])
            nc.scalar.dma_start(p_d[:], po[:])


def _shard_inputs(outputs, W, w):
    f16 = np.float16
    outputs = np.asarray(outputs, dtype=np.float32)
    W = np.asarray(W, dtype=np.float32)
    w = np.asarray(w, dtype=np.float32)
    wt = np.ascontiguousarray(w.reshape(N_OCHUNK, 128).T).astype(f16)
    in_maps = []
    for k in range(N_CORES):
        cols = slice(HS * k, HS * (k + 1))
        xk = outputs[:, cols]  # [8192, 512]
        # [(g, hc*128+p), s] = X[1024g + s, 512k + 128hc + p]
        xt = (
            np.ascontiguousarray(xk.reshape(NG, SG, HS).transpose(0, 2, 1))
            .reshape(NG * HS, SG)
            .astype(f16)
        )
        in_maps.append(
            {
                "x": xt,
                "wc": np.ascontiguousarray(W[:, cols]).astype(f16),
                "wt": wt,
            }
        )
    return in_maps


def _run(outputs, W, w, trace=False, trace_cores=None):
    from concourse.bass_utils import run_bass_kernel_spmd

    if "nc" not in _CACHE:
        _CACHE["nc"] = _build_nc()
    nc = _CACHE["nc"]
    in_maps = _shard_inputs(outputs, W, w)
    res = run_bass_kernel_spmd(
        nc, in_maps, list(range(N_CORES)), trace=trace, trace_cores=trace_cores
    )
    # p[p, c] = softmax(e)[64p + c]; row-major reshape restores s-order
    full = np.asarray(res.results[0]["p"]).reshape(1, 1, S).astype(np.float32)
    return full, res


def kernel(outputs, W, b, w):
    out, _ = _run(outputs, W, w, trace=False)
    return out


def kernel_traced(outputs, W, b, w, trace_cores=None):
    out, res = _run(outputs, W, w, trace=True, trace_cores=trace_cores)
    return out, res


# revision 14
# speedup vs baseline: 1.4195x; 1.1856x over previous
"""Trainium2 Bass kernel for nn_Attn_76424648065726.

Computes softmax(einsum('so,o->s', outputs @ W.T + b, w)) reshaped to
[1, 1, S].

Math: (outputs @ W.T + b) @ w == outputs @ (W.T @ w) + dot(b, w), and the
scalar dot(b, w) cancels inside softmax.  So the kernel computes
softmax(outputs @ v) with v = W.T @ w — turning the [S,H2]x[H2,H2] matmul
into a memory-bound matvec pipeline.

Sharding (8 cores, hidden-dim parallel): core k owns columns
[512k, 512k+512) of both W and outputs (no cross-core data needed until
the energies are summed).
  phase 1: v_k = W[:, cols_k].T @ w            (PE, PSUM accumulate)
  PE-transpose v_k [1,512] -> vt [128,4]
  phase 2: e_k[s] = outputs[s, cols_k] @ v_k   (PE matvec on X^T tiles)
  ReduceScatter(add): core k gets summed energies for s in
    [1024k, 1024k+1024)
  local max/exp/sum, AllGather of the (max, expsum) pairs (8 B/rank),
  rescale own 1024 values, output own chunk; host concatenates.

outputs/W/w are staged to fp16 on the host (halves HBM traffic, faster
PE).  All accumulation is fp32 (PSUM / ACT accumulator).  X is staged
host-side in a transposed, DMA-friendly tiled layout so phase 2 runs on
the otherwise-idle PE instead of DVE+ACT.
"""

import numpy as np

N_CORES = 8
S = 8192
H2 = 4096
HS = H2 // N_CORES  # 512 columns of W / outputs per core
N_OCHUNK = H2 // 128  # 32 contraction chunks for v
WCPG = 4  # o-chunks per W tile (DMA batch)
NHC = HS // 128  # 4 h-chunks per core
NG = 8  # X s-groups per core (1024 s each)
SG = S // NG  # 1024
SC = S // N_CORES  # 1024 output chunk per core

_CACHE = {}


def _build_nc(enable_asserts=False, debug_taps=False):
    import concourse.bass as bass
    import concourse.tile as tile
    from concourse import bacc, mybir

    nc = bacc.Bacc(
        "TRN2",
        target_bir_lowering=False,
        debug=False,
        enable_asserts=enable_asserts,
        num_devices=N_CORES,
    )
    fp32 = mybir.dt.float32
    f16 = mybir.dt.float16
    # x: X[:, cols_k] transposed + grouped:
    #   x[(g*4 + hc)*128 + p, s] = X[1024g + s, 512k + 128hc + p]
    x_d = nc.dram_tensor("x", [NG * HS, SG], f16, kind="ExternalInput").ap()
    wc_d = nc.dram_tensor("wc", [H2, HS], f16, kind="ExternalInput").ap()
    wt_d = nc.dram_tensor("wt", [128, N_OCHUNK], f16, kind="ExternalInput").ap()
    p_d = nc.dram_tensor("p", [128, S // 128], fp32, kind="ExternalOutput").ap()
    dbg = None
    if debug_taps:
        dbg = {
            "dbg_vt": nc.dram_tensor("dbg_vt", [128, NHC], f16,
                                     kind="ExternalOutput").ap(),
            "dbg_e": nc.dram_tensor("dbg_e", [1, S], fp32,
                                    kind="ExternalOutput").ap(),
            "dbg_ef": nc.dram_tensor("dbg_ef", [128, 8], fp32,
                                     kind="ExternalOutput").ap(),
        }

    with tile.TileContext(nc) as tc:
        _body(tc, x_d, wc_d, wt_d, p_d, dbg=dbg)
    nc.compile()
    return nc


def _body(tc, x_d, wc_d, wt_d, p_d, dbg=None):
    import concourse.bass as bass
    from concourse import bass_isa, mybir

    nc = tc.nc
    fp32 = mybir.dt.float32
    f16 = mybir.dt.float16
    ts = bass.ts

    from contextlib import ExitStack

    with ExitStack() as ctx:
        wpool = ctx.enter_context(tc.tile_pool(name="wpool", bufs=8))
        xpool = ctx.enter_context(tc.tile_pool(name="xpool", bufs=8))
        small = ctx.enter_context(tc.tile_pool(name="small", bufs=1))
        dram = ctx.enter_context(tc.tile_pool(name="dram", bufs=1, space="DRAM"))

        # w, pre-transposed on host to [128, 32]: wt[p, c] = w[c*128 + p]
        wt_sb = small.tile([128, N_OCHUNK], f16)
        nc.scalar.dma_start(wt_sb[:], wt_d[:])

        # Dummy 4-byte AllReduce fired immediately: absorbs the cross-core
        # launch/CC-stream-init skew concurrently with the DMA stream, so
        # the real energy AllReduce later finds the ranks aligned.
        warm0 = small.tile([1, 1], fp32)
        nc.vector.memset(warm0[:], 0.0)
        w0_dr = dram.tile([1, 1], fp32)
        w0_out = dram.tile([1, 1], fp32)
        nc.scalar.dma_start(w0_dr[:], warm0[:])
        nc.gpsimd.collective_compute(
            "AllReduce",
            mybir.AluOpType.add,
            replica_groups=[list(range(N_CORES))],
            ins=[w0_dr.opt()],
            outs=[w0_out.opt()],
        )

        # constants for the PE-based partition reduce/broadcast in the
        # softmax tail; built on idle engines during the DMA shadow.
        from concourse import masks

        id128 = small.tile([128, 128], fp32)
        masks.make_identity(nc, id128[:])
        ones_r = small.tile([1, 128], fp32)
        nc.vector.memset(ones_r[:], 1.0)
        ones_c = small.tile([128, 1], fp32)
        nc.vector.memset(ones_c[:], 1.0)

        # All streaming DMAs go on the sync HWDGE ring in FIFO order: W
        # first (it gates phase 1), then X.  One ring keeps HBM busy;
        # both rings share the same 16 SDMA engines so spreading gains
        # nothing.  ~0.5-1MiB slices amortize ring overhead.
        wtiles = []
        for g in range(N_OCHUNK // WCPG):
            wtile = wpool.tile([128, WCPG, HS], f16)
            nc.sync.dma_start(
                wtile[:],
                wc_d[ts(g, 128 * WCPG), :].rearrange("(c p) j -> p c j", p=128),
            )
            wtiles.append(wtile)

        xtiles = []
        for g in range(NG):
            xt = xpool.tile([128, NHC, SG], f16)
            nc.sync.dma_start(
                xt[:],
                x_d[ts(g, NHC * 128), :].rearrange("(h p) s -> p h s", p=128),
            )
            xtiles.append(xt)

        with ExitStack() as p1ctx:
            # PE warmup: the HAM throttles a cold PE to 1.2 GHz; ~10us of
            # dummy matmuls on memset data while W streams in gets the real
            # matmuls the 2.4 GHz rate.
            wu_pool = p1ctx.enter_context(tc.tile_pool(name="wu_pool", bufs=1))
            wu_psum = p1ctx.enter_context(
                tc.tile_pool(name="wu_psum", bufs=1, space="PSUM")
            )
            vpsum = p1ctx.enter_context(
                tc.tile_pool(name="vpsum", bufs=1, space="PSUM")
            )
            vtpsum = p1ctx.enter_context(
                tc.tile_pool(name="vtpsum", bufs=1, space="PSUM")
            )
            wu_lhs = wu_pool.tile([128, 1], f16)
            wu_rhs = wu_pool.tile([128, HS], f16)
            nc.vector.memset(wu_lhs[:], 0.0)
            nc.vector.memset(wu_rhs[:], 0.0)
            wu_ps = wu_psum.tile([1, HS], fp32)
            for i in range(10):
                nc.tensor.matmul(
                    wu_ps[:], lhsT=wu_lhs[:], rhs=wu_rhs[:], start=True, stop=True
                )
            # short (N=128) dummies bridge the gap until W arrives — a ~2us
            # PE idle re-throttles the HAM.
            for i in range(20):
                nc.tensor.matmul(
                    wu_ps[:, :128], lhsT=wu_lhs[:], rhs=wu_rhs[:, :128],
                    start=True, stop=True,
                )

            # ---- phase 1: v = W_k.T @ w  ([1, HS] accumulated in PSUM) ----
            v_ps = vpsum.tile([1, HS], fp32)
            for c in range(N_OCHUNK):
                nc.tensor.matmul(
                    v_ps[:],
                    lhsT=wt_sb[:, c : c + 1],
                    rhs=wtiles[c // WCPG][:, c % WCPG, :],
                    start=(c == 0),
                    stop=(c == N_OCHUNK - 1),
                )

            v_row = small.tile([1, HS], fp32)
            nc.vector.tensor_copy(v_row[:], v_ps[:])

            # PE-transpose v [1, 512] -> vt [128, 4] (vt[p, hc] = v[128hc+p])
            # fp32 keeps the PSUM column slices 4-byte aligned.
            id1 = wu_pool.tile([1, 1], fp32)
            nc.vector.memset(id1[:], 1.0)
            vt_ps = vtpsum.tile([128, NHC], fp32)
            for hc in range(NHC):
                nc.tensor.transpose(
                    vt_ps[:, hc : hc + 1], v_row[:, ts(hc, 128)], id1[:]
                )
            vt = small.tile([128, NHC], f16)
            nc.vector.tensor_copy(vt[:], vt_ps[:])
            if dbg is not None:
                nc.scalar.dma_start(dbg["dbg_vt"][:], vt[:])

        # ---- phase 2: partial energies e[s] = X[s, cols_k] @ v_k on PE ----
        # e lives as [1, 8192] fp32 on partition 0.
        e_sb = small.tile([1, S], fp32)
        with ExitStack() as p2ctx:
            epool = p2ctx.enter_context(
                tc.tile_pool(name="epsum", bufs=8, space="PSUM")
            )
            for g in range(NG):
                for half in range(2):
                    e_ps = epool.tile([1, 512], fp32)
                    for hc in range(NHC):
                        nc.tensor.matmul(
                            e_ps[:],
                            lhsT=vt[:, hc : hc + 1],
                            rhs=xtiles[g][:, hc, ts(half, 512)],
                            start=(hc == 0),
                            stop=(hc == NHC - 1),
                        )
                    nc.vector.tensor_copy(
                        e_sb[:, g * SG + half * 512 : g * SG + half * 512 + 512],
                        e_ps[:],
                    )

        # ---- single AllReduce of the 32 KiB energy vector ----
        e_dr = dram.tile([1, S], fp32)
        e_red = dram.tile([1, S], fp32)
        nc.scalar.dma_start(e_dr[:], e_sb[:])
        nc.gpsimd.collective_compute(
            "AllReduce",
            mybir.AluOpType.add,
            replica_groups=[list(range(N_CORES))],
            ins=[e_dr.opt()],
            outs=[e_red.opt()],
        )
        # land the summed energies partition-spread: ef128[p, c] = e[64p + c]
        ef128 = small.tile([128, S // 128], fp32)
        nc.scalar.dma_start(
            ef128[:], e_red[:].rearrange("o (p c) -> (o p) c", p=128)
        )
        if dbg is not None:
            nc.scalar.dma_start(dbg["dbg_e"][:], e_sb[:])
            nc.scalar.dma_start(dbg["dbg_ef"][:], ef128[:, :8])

        # ---- softmax over all S values (redundant on every core) ----
        # partition reduces / broadcasts run on the idle PE via matmuls.
        with ExitStack() as tctx:
            tpsum = tctx.enter_context(
                tc.tile_pool(name="tpsum", bufs=1, space="PSUM")
            )
            m1 = small.tile([128, 1], fp32)
            nc.vector.tensor_reduce(
                m1[:], ef128[:], axis=mybir.AxisListType.X, op=mybir.AluOpType.max
            )
            mT_ps = tpsum.tile([1, 128], fp32)
            nc.tensor.transpose(mT_ps[:], m1[:], id128[:])
            mT = small.tile([1, 128], fp32)
            nc.vector.tensor_copy(mT[:], mT_ps[:])
            M = small.tile([1, 1], fp32)
            nc.vector.tensor_reduce(
                M[:], mT[:], axis=mybir.AxisListType.X, op=mybir.AluOpType.max
            )
            nM = small.tile([1, 1], fp32)
            nc.scalar.mul(nM[:], M[:], -1.0)
            nmb_ps = tpsum.tile([128, 1], fp32)
            nc.tensor.matmul(
                nmb_ps[:], lhsT=ones_r[:], rhs=nM[:], start=True, stop=True
            )
            nmb = small.tile([128, 1], fp32)
            nc.vector.tensor_copy(nmb[:], nmb_ps[:])

            pexp = small.tile([128, S // 128], fp32)
            s1 = small.tile([128, 1], fp32)
            nc.scalar.activation(
                pexp[:],
                ef128[:],
                mybir.ActivationFunctionType.Exp,
                bias=nmb[:],
                scale=1.0,
                accum_out=s1[:],
            )
            z_ps = tpsum.tile([1, 1], fp32)
            nc.tensor.matmul(
                z_ps[:], lhsT=s1[:], rhs=ones_c[:], start=True, stop=True
            )
            z = small.tile([1, 1], fp32)
            nc.vector.tensor_copy(z[:], z_ps[:])
            rz = small.tile([1, 1], fp32)
            nc.vector.reciprocal(rz[:], z[:])
            rzb_ps = tpsum.tile([128, 1], fp32)
            nc.tensor.matmul(
                rzb_ps[:], lhsT=ones_r[:], rhs=rz[:], start=True, stop=True
            )
            rzb = small.tile([128, 1], fp32)
            nc.vector.tensor_copy(rzb[:], rzb_ps[:])

            po = small.tile([128, S // 128], fp32)
            nc.scalar.mul(po[:], pexp[:], rzb[:])
            nc.scalar.dma_start(p_d[:], po[:])


def _shard_inputs(outputs, W, w):
    f16 = np.float16
    outputs = np.asarray(outputs, dtype=np.float32)
    W = np.asarray(W, dtype=np.float32)
    w = np.asarray(w, dtype=np.float32)
    wt = np.ascontiguousarray(w.reshape(N_OCHUNK, 128).T).astype(f16)
    in_maps = []
    for k in range(N_CORES):
        cols = slice(HS * k, HS * (k + 1))
        xk = outputs[:, cols]  # [8192, 512]
        # [(g, hc*128+p), s] = X[1024g + s, 512k + 128hc + p]
        xt = (
            np.ascontiguousarray(xk.reshape(NG, SG, HS).transpose(0, 2, 1))
            .reshape(NG * HS, SG)
            .astype(f16)
        )
        in_maps.append(
            {
                "x": xt,
                "wc": np.ascontiguousarray(W[:, cols]).astype(f16),
                "wt": wt,
            }
        )
    return in_maps


def _run(outputs, W, w, trace=False, trace_cores=None):
    from concourse.bass_utils import run_bass_kernel_spmd

    if "nc" not in _CACHE:
        _CACHE["nc"] = _build_nc()
    nc = _CACHE["nc"]
    in_maps = _shard_inputs(outputs, W, w)
    res = run_bass_kernel_spmd(
        nc, in_maps, list(range(N_CORES)), trace=trace, trace_cores=trace_cores
    )
    # p[p, c] = softmax(e)[64p + c]; row-major reshape restores s-order
    full = np.asarray(res.results[0]["p"]).reshape(1, 1, S).astype(np.float32)
    return full, res


def kernel(outputs, W, b, w):
    out, _ = _run(outputs, W, w, trace=False)
    return out


def kernel_traced(outputs, W, b, w, trace_cores=None):
    out, res = _run(outputs, W, w, trace=True, trace_cores=trace_cores)
    return out, res
